# revision 24
# baseline (speedup 1.0000x reference)
"""NTM forward kernel for 8 Trainium2 NeuronCores (data-parallel over batch).

The reference NTM runs a single step from zero state: memory == 0, so the
read vector is exactly zero and the whole addressing path drops out.  What
remains is:
    gates = x @ W_ih.T + (b_ih + b_hh)        (f-gate unused: c0 == 0)
    h     = sigmoid(o) * tanh(sigmoid(i) * tanh(g))
    out   = h @ W_out[:, :H].T + b_out

All matmul operands are fp32r (fp32 with 11-bit mantissa): bf16 matmuls
trigger the compiler's fast-weight-load path whose LDWEIGHTS does NOT hide
behind the previous matmul (measured 259 ns/MM vs fp32r's 229), and walrus
rejects mixing 32-bit with 16-bit matmul inputs.  x is shipped over DMA as
bf16 (half the bytes on the critical ramp) and cast to fp32r by the
otherwise-idle vector engine.  Six PSUM banks accumulate (i,g,o) x (bc0,
bc1) concurrently in phase 1.  End-to-end error vs fp32 is ~2e-3
L2-relative (bf16 quantization of x only).
"""
from contextlib import ExitStack

import ml_dtypes
import numpy as np

import concourse.bass as bass
import concourse.tile as tile
from concourse import bacc, mybir
from concourse.bass_utils import run_bass_kernel_spmd

NCORES = 8
B, D, H = 8192, 1024, 2048
BL = B // NCORES          # 1024 batch rows per core
P = 128
NKD = D // P              # 8  k-tiles over input dim
NT = H // P               # 16 hh row-blocks (one i/g/o triple each)
NK2 = H // P              # 16 k-tiles over hidden dim
NBC = BL // 512           # 2  batch chunks of 512
WARMUP_MMS = 8
BF16 = mybir.dt.bfloat16
F32R = mybir.dt.float32r
F32 = mybir.dt.float32
ACT_SIG = mybir.ActivationFunctionType.Sigmoid
ACT_TANH = mybir.ActivationFunctionType.Tanh
ACT_COPY = mybir.ActivationFunctionType.Copy


def _round_fp32r(a: np.ndarray) -> np.ndarray:
    """RNE-round fp32 to the fp32r grid (11 mantissa bits, low 12 bits 0)."""
    b = np.ascontiguousarray(a, dtype=np.float32).view(np.uint32)
    r = (b + np.uint32(0x7FF) + ((b >> np.uint32(12)) & np.uint32(1))) & np.uint32(
        0xFFFFF000
    )
    return r.view(np.float32)


def _build_program():
    nc = bacc.Bacc("TRN2", target_bir_lowering=False, debug=False,
                   num_devices=NCORES)

    x_d = nc.dram_tensor("x", [P, NKD, BL], BF16, kind="ExternalInput").ap()
    w1_d = nc.dram_tensor("w1", [NT, P, 3 * NKD * P], F32R,
                          kind="ExternalInput").ap()
    w2_d = nc.dram_tensor("w2", [8, P, NK2 * P], F32R,
                          kind="ExternalInput").ap()
    bias_d = nc.dram_tensor("bias", [P, 3 * NT], F32, kind="ExternalInput").ap()
    out_d = nc.dram_tensor("outT", [D, BL], F32, kind="ExternalOutput").ap()

    with tile.TileContext(nc) as tc, ExitStack() as ctx:
        xpool = ctx.enter_context(tc.tile_pool(name="xpool", bufs=1))
        hpool = ctx.enter_context(tc.tile_pool(name="hpool", bufs=1))
        bpool = ctx.enter_context(tc.tile_pool(name="bpool", bufs=1))
        w1pool = ctx.enter_context(tc.tile_pool(name="w1pool", bufs=3))
        w2pool = ctx.enter_context(tc.tile_pool(name="w2pool", bufs=3))
        apool = ctx.enter_context(tc.tile_pool(name="apool", bufs=2))
        opool = ctx.enter_context(tc.tile_pool(name="opool", bufs=2))
        ps1 = ctx.enter_context(tc.tile_pool(name="ps1", bufs=6, space="PSUM"))
        ps2 = ctx.enter_context(tc.tile_pool(name="ps2", bufs=2, space="PSUM"))

        # Critical path to the first matmul: w1[t=0] k-chunks on the sync
        # HWDGE ring; x k-tiles arrive as bf16 (half the bytes) on the
        # scalar/gpsimd rings and are cast to fp32r by the (idle) vector
        # engine, double-buffered through 4 staging tiles.
        w1_sb0 = w1pool.tile([P, 3 * NKD * P], F32R, tag="w1")
        nc.sync.dma_start(w1_sb0[:, 0:384], w1_d[0][:, 0:384])
        x_sb = [xpool.tile([P, BL], F32R, tag=f"x{k}", name=f"x{k}")
                for k in range(NKD)]
        x_stg = [xpool.tile([P, BL], BF16, tag=f"xs{j}", name=f"xs{j}")
                 for j in range(4)]
        for k in range(NKD):
            eng = nc.gpsimd if k % 2 else nc.scalar
            stg = x_stg[2 * (k % 2) + (k // 2) % 2]
            eng.dma_start(stg[:], x_d[:, k, :])
            nc.vector.tensor_copy(x_sb[k][:], stg[:])
        for k in range(1, NKD):
            nc.sync.dma_start(w1_sb0[:, k * 384:(k + 1) * 384],
                              w1_d[0][:, k * 384:(k + 1) * 384])
        bias_sb = bpool.tile([P, 3 * NT], F32)
        nc.gpsimd.dma_start(bias_sb[:], bias_d[:])
        h_sb = hpool.tile([P, NK2 * BL], F32R)          # [hh_p, k2*BL + b]

        if WARMUP_MMS:
            # Warm the PE clock (HAM un-throttles after ~3.4us of activity)
            # while the prologue DMAs are still in flight.
            warm_sb = bpool.tile([P, 512], BF16)
            nc.vector.memset(warm_sb[:], 0.0)
            warm_ps = ps1.tile([P, 512], F32, tag="ps1")
            for _ in range(WARMUP_MMS):
                nc.tensor.matmul(warm_ps[:], warm_sb[:, 0:P], warm_sb[:],
                                 start=True, stop=True)

        # ---- phase 1: gates + activations -> h ----
        for t in range(NT):
            if t == 0:
                w1_sb = w1_sb0
            else:
                w1_sb = w1pool.tile([P, 3 * NKD * P], F32R, tag="w1")
                nc.sync.dma_start(w1_sb[:], w1_d[t])

            # Six PSUM banks accumulate (i,g,o) x (bc0,bc1) concurrently;
            # each stationary weight tile serves both batch halves
            # back-to-back, so LDWEIGHTS runs once per two matmuls.
            ps = [ps1.tile([P, 512], F32, tag="ps1", name=f"ps1_{j}")
                  for j in range(6)]
            for k in range(NKD):
                for gi in range(3):
                    for bc in range(NBC):
                        nc.tensor.matmul(
                            ps[gi * NBC + bc][:],
                            w1_sb[:, (k * 3 + gi) * P:(k * 3 + gi + 1) * P],
                            x_sb[k][:, bc * 512:(bc + 1) * 512],
                            start=(k == 0), stop=(k == NKD - 1),
                        )
            for bc in range(NBC):
                gate_sb = []
                for gi, func in ((0, ACT_SIG), (1, ACT_TANH), (2, ACT_SIG)):
                    bias_ap = bias_sb[:, 3 * t + gi:3 * t + gi + 1]
                    g_sb = apool.tile([P, 512], F32, tag=f"act{gi}")
                    nc.scalar.activation(g_sb[:], ps[gi * NBC + bc][:], func,
                                         bias=bias_ap)
                    gate_sb.append(g_sb)
                c_sb = apool.tile([P, 512], F32, tag="c")
                nc.vector.tensor_mul(c_sb[:], gate_sb[0][:], gate_sb[1][:])
                tc_sb = apool.tile([P, 512], F32, tag="tanh_c")
                nc.scalar.activation(tc_sb[:], c_sb[:], ACT_TANH)
                h_slice = h_sb[:, t * BL + bc * 512:t * BL + (bc + 1) * 512]
                nc.vector.tensor_mul(h_slice, gate_sb[2][:], tc_sb[:])

        # ---- phase 2: outT = W_outT.T @ h ----
        # w2 streams through a 3-deep pool on the sync ring (queued behind
        # the w1 loads, so the first tiles land well before phase 2).
        # PSUM->SBUF copies run on the idle vector engine; out stores go on
        # the gpsimd/scalar rings and never block a load.  The last tile
        # (nt=7) is unpaired and its stores split across rings to shorten
        # the end-of-kernel critical chain.
        for nt in range(8):
            w2_sb = w2pool.tile([P, NK2 * P], F32R, tag="w2")
            nc.sync.dma_start(w2_sb[:], w2_d[nt])
            if nt < 7:
                ps = [ps2.tile([P, 512], F32, tag="ps2", name=f"ps2_{bc}")
                      for bc in range(NBC)]
                for k2 in range(NK2):
                    for bc in range(NBC):
                        nc.tensor.matmul(
                            ps[bc][:],
                            w2_sb[:, k2 * P:(k2 + 1) * P],
                            h_sb[:, k2 * BL + bc * 512:
                                 k2 * BL + (bc + 1) * 512],
                            start=(k2 == 0), stop=(k2 == NK2 - 1),
                        )
                for bc in range(NBC):
                    o_sb = opool.tile([P, 512], F32, tag=f"osb{bc}")
                    nc.vector.tensor_copy(o_sb[:], ps[bc][:])
                    eng = nc.gpsimd if bc == 0 else nc.scalar
                    eng.dma_start(
                        out_d[nt * P:(nt + 1) * P, bc * 512:(bc + 1) * 512],
                        o_sb[:],
                    )
            else:
                for bc in range(NBC):
                    psl = ps2.tile([P, 512], F32, tag="ps2", name="ps2_l")
                    for k2 in range(NK2):
                        nc.tensor.matmul(
                            psl[:],
                            w2_sb[:, k2 * P:(k2 + 1) * P],
                            h_sb[:, k2 * BL + bc * 512:
                                 k2 * BL + (bc + 1) * 512],
                            start=(k2 == 0), stop=(k2 == NK2 - 1),
                        )
                    o_sb = opool.tile([P, 512], F32, tag=f"osb{bc}")
                    for half, eng in ((0, nc.gpsimd), (1, nc.scalar)):
                        sl = slice(half * 256, (half + 1) * 256)
                        nc.vector.tensor_copy(o_sb[:, sl], psl[:, sl])
                        eng.dma_start(
                            out_d[nt * P:(nt + 1) * P,
                                  bc * 512 + half * 256:
                                  bc * 512 + (half + 1) * 256],
                            o_sb[:, sl],
                        )

    nc.compile()
    return nc


_CACHE: dict = {}


def _get_program():
    if "nc" not in _CACHE:
        _CACHE["nc"] = _build_program()
    return _CACHE["nc"]


def _prep_inputs(x, W_ih, b_ih, b_hh, W_out):
    """Host-side reshape/round. Returns per-core input maps."""
    # gate rows: torch order i, f, g, o; f unused.
    hh = np.arange(H)
    row_map = np.empty(3 * H, dtype=np.int64)
    for t in range(NT):
        for gi, rows in enumerate((hh, 2 * H + hh, 3 * H + hh)):
            row_map[t * 384 + gi * P:t * 384 + (gi + 1) * P] = \
                rows[t * P:(t + 1) * P]

    W_sel = W_ih[row_map]                                   # [6144, 1024]
    # w1[t, p_d, k*384 + gi*128 + jj] = W_sel[t*384+gi*128+jj, k*128+p_d]
    w1 = W_sel.reshape(NT, 3, P, NKD, P).transpose(0, 4, 3, 1, 2) \
        .reshape(NT, P, 3 * NKD * P)
    w1 = _round_fp32r(w1)

    bias_sel = (b_ih + b_hh)[row_map].astype(np.float32)    # [6144]
    bias = np.ascontiguousarray(bias_sel.reshape(3 * NT, P).T)  # [128, 48]

    # w2[n_tile, p_hh, k2*128 + m] = W_out[n_tile*128+m, k2*128+p_hh]
    w2 = W_out[:, :H].reshape(8, P, NK2, P).transpose(0, 3, 2, 1) \
        .reshape(8, P, NK2 * P)
    w2 = _round_fp32r(w2)

    in_maps = []
    for c in range(NCORES):
        xc = x[c * BL:(c + 1) * BL]                         # [1024 b, 1024 d]
        # x_dev[p_d, k, b] = xc[b, k*128 + p_d]
        x_dev = np.ascontiguousarray(
            xc.reshape(BL, NKD, P).transpose(2, 1, 0)
        ).astype(ml_dtypes.bfloat16)
        in_maps.append({"x": x_dev, "w1": w1, "w2": w2, "bias": bias})
    return in_maps


def kernel(x, W_ih, b_ih, b_hh, W_read, b_read, W_out, b_out, **_ignored):
    x = np.asarray(x, dtype=np.float32)
    W_ih = np.asarray(W_ih, dtype=np.float32)
    b_ih = np.asarray(b_ih, dtype=np.float32)
    b_hh = np.asarray(b_hh, dtype=np.float32)
    W_out = np.asarray(W_out, dtype=np.float32)
    b_out = np.asarray(b_out, dtype=np.float32)

    nc = _get_program()
    in_maps = _prep_inputs(x, W_ih, b_ih, b_hh, W_out)
    res = run_bass_kernel_spmd(nc, in_maps, list(range(NCORES)))

    out = np.empty((B, D), dtype=np.float32)
    for c in range(NCORES):
        out[c * BL:(c + 1) * BL] = res.results[c]["outT"].T
    out += b_out[None, :]
    return out
